# revision 14
# baseline (speedup 1.0000x reference)
"""Trainium2 multi-head attention kernel (8 NeuronCores).

Sharding: 2 (batch) x 4 (head-group) grid. Core c handles batch b=c//4 and
heads [4g, 4g+4) where g=c%4 (d_model slice of 256).

Per core:
  1. Q^T,K^T projections [256,2048] (fp32r) and V [2048,256] (fp16) for its
     heads, contraction d_model=1024.
  2. Attention: scores^T = Kh^T.T @ Qh^T per head (two heads packed onto PE
     row-groups via tile_position), exp via ScalarE with fused 1/8 scale, AV
     in fp16 with a ones-augmented V so the softmax denominators fall out of
     the same matmul, then an accurate-reciprocal normalize.
  3. Partial output projection, computed transposed (out^T = WoT.T @ attn^T)
     so the weight stays stationary: [1024, 2048] partial sum in fp16.
Host: all inputs are pre-transposed/sliced per core; the 4 partial outputs of
each batch are summed on host (the unshard step of this tensor-parallel
layout), transposed back and concatenated over batch.
"""
import os
import sys

import numpy as np

for _p in ("/opt/trn_rl_repo", "/root/.axon_site/_ro/trn_rl_repo"):
    if _p not in sys.path:
        sys.path.append(_p)

import concourse.bacc as bacc
import concourse.mybir as mybir
import concourse.tile as tile
from concourse.bass_utils import run_bass_kernel_spmd

F32 = mybir.dt.float32
F32R = mybir.dt.float32r
F16 = mybir.dt.float16

B, S, D, H, DK = 2, 2048, 1024, 16, 64
NC_ = 8
HG = D // 4          # 256: d_model slice per core
KT_D = D // 128      # 8 contraction tiles for projections
KT_S = S // 128      # 16 sequence tiles
QC = S // 512        # 4 query chunks of 512
AF = mybir.ActivationFunctionType

FP16_SCORES = os.environ.get("FP16_SCORES") == "1"
SCORE_DT = F16 if FP16_SCORES else F32R

if os.environ.get("LDW_OPT") == "1":
    import concourse.bass_utils as _bu

    if not getattr(_bu, "_ldw_opt_patched", False):
        _orig_run_command = _bu.run_command

        def _run_command_ldw(cmd, **kw):
            cmd = [c.replace("--enable-ldw-opt=false", "--enable-ldw-opt=true")
                   if isinstance(c, str) else c for c in cmd]
            return _orig_run_command(cmd, **kw)

        _bu.run_command = _run_command_ldw
        _bu._ldw_opt_patched = True


def build_nc():
    nc = bacc.Bacc("TRN2", target_bir_lowering=False, debug=False, num_devices=NC_)

    xqT = nc.dram_tensor("xqT", [D, S], F32R, kind="ExternalInput").ap()
    xkT = nc.dram_tensor("xkT", [D, S], F32R, kind="ExternalInput").ap()
    xvT = nc.dram_tensor("xvT", [D, S], F16, kind="ExternalInput").ap()
    wqT = nc.dram_tensor("wqT", [D, HG], F32R, kind="ExternalInput").ap()
    wkT = nc.dram_tensor("wkT", [D, HG], F32R, kind="ExternalInput").ap()
    wvT = nc.dram_tensor("wvT", [D, HG], F16, kind="ExternalInput").ap()
    bqv = nc.dram_tensor("bqv", [128, 4], F32, kind="ExternalInput").ap()
    bvb = nc.dram_tensor("bvb", [128, HG], F32, kind="ExternalInput").ap()
    woT = nc.dram_tensor("woT", [HG, D], F16, kind="ExternalInput").ap()
    bob = nc.dram_tensor("bob", [128, KT_D], F32, kind="ExternalInput").ap()
    outT = nc.dram_tensor("outT", [D, S], F16, kind="ExternalOutput").ap()

    with tile.TileContext(nc) as tc:
        with (
            tc.tile_pool(name="const", bufs=1) as cpool,
            tc.tile_pool(name="proj", bufs=1) as ppool,
            tc.tile_pool(name="xs", bufs=4) as xpool,
            tc.tile_pool(name="exp", bufs=5) as epool,
            tc.tile_pool(name="nrm", bufs=2) as npool,
            tc.tile_pool(name="ost", bufs=3) as opool,
            tc.tile_pool(name="psC", bufs=1, space="PSUM") as psC,
        ):
            # ---- persistent tiles ----
            wq_t = cpool.tile([128, KT_D, HG], F32R)
            wk_t = cpool.tile([128, KT_D, HG], F32R)
            wv_t = cpool.tile([128, KT_D, HG], F16)
            bqv_t = cpool.tile([128, 4], F32)
            nc.sync.dma_start(bqv_t[:], bqv[:])
            bvb_t = cpool.tile([128, HG], F32)
            bob_t = cpool.tile([128, KT_D], F32)
            wo_t = cpool.tile([128, 2, D], F16)

            qT = ppool.tile([128, 2, S], SCORE_DT)  # [o-part, Mtile, t]
            kT = ppool.tile([128, 2, S], SCORE_DT)
            vS = ppool.tile([128, KT_S, 4 * 128], F16)  # [t-part, t-tile, head*65]
            aoT = ppool.tile([128, 2, S], F16)  # normalized attn out^T

            # ---- phase A: Q^T then K^T projections ----
            # 4 accumulators live (2 Mtiles x 2 qc of a pair), borrowed from
            # the attention pool's tags; lhsT reused across the qc pair.
            for (w_t, w_d, dst, xsrc, xtag, boff) in (
                (wq_t, wqT, qT, xqT, "xq_c", 0),
                (wk_t, wkT, kT, xkT, "xk_c", 2),
            ):
                for kt in range(KT_D):
                    nc.sync.dma_start(
                        w_t[:, kt, :], w_d[kt * 128:(kt + 1) * 128, :])
                for qp in range(QC // 2):
                    acc = [
                        psC.tile([128, 512], F32, name=f"acc{m}{j}",
                                 tag=f"av{j}", bufs=2)
                        for m in range(2) for j in range(2)
                    ]
                    for kt in range(KT_D):
                        x_c = xpool.tile([128, 1024], F32R, name="x_c",
                                         tag=xtag)
                        nc.sync.dma_start(
                            x_c[:], xsrc[kt * 128:(kt + 1) * 128,
                                         qp * 1024:(qp + 1) * 1024])
                        for m in range(2):
                            for j in range(2):
                                nc.tensor.matmul(
                                    acc[m * 2 + j][:],
                                    w_t[:, kt, m * 128:(m + 1) * 128],
                                    x_c[:, j * 512:(j + 1) * 512],
                                    start=(kt == 0), stop=(kt == KT_D - 1))
                    for m in range(2):
                        for j in range(2):
                            qc = qp * 2 + j
                            nc.scalar.activation(
                                dst[:, m, qc * 512:(qc + 1) * 512],
                                acc[m * 2 + j][:],
                                AF.Identity,
                                bias=bqv_t[:, boff + m:boff + m + 1])

            # ---- phase B: V projection (token-major layout, fp16) ----
            # ones columns of vS (softmax denominator trick): fill whole
            # tile with 1.0 once; V writes below overwrite all but col 64.
            nc.gpsimd.memset(vS[:], 1.0)
            for kt in range(KT_D):
                nc.sync.dma_start(
                    wv_t[:, kt, :], wvT[kt * 128:(kt + 1) * 128, :])
            nc.sync.dma_start(bvb_t[:], bvb[:])
            # one accumulator per PSUM bank (interleaved chains in a single
            # bank corrupt each other: start=True clears the whole bank)
            for tc2 in range(KT_S // 2):
                psv = [psC.tile([128, HG], F32, name=f"psv{t}",
                                tag=f"av{t}", bufs=2) for t in range(2)]
                for kt in range(KT_D):
                    xv_c = xpool.tile([128, 256], F16, name="xv_c", tag="xv_c")
                    nc.sync.dma_start(
                        xv_c[:], xvT[kt * 128:(kt + 1) * 128,
                                     tc2 * 256:(tc2 + 1) * 256])
                    for t in range(2):
                        nc.tensor.matmul(
                            psv[t][:],
                            xv_c[:, t * 128:(t + 1) * 128],
                            wv_t[:, kt, :], start=(kt == 0),
                            stop=(kt == KT_D - 1))
                for t in range(2):
                    tg = tc2 * 2 + t
                    for h in range(4):
                        nc.vector.tensor_tensor(
                            vS[:, tg, h * 128:h * 128 + 64],
                            psv[t][:, h * 64:(h + 1) * 64],
                            bvb_t[:, h * 64:(h + 1) * 64],
                            op=mybir.AluOpType.add)

            # ---- phase C: attention ----
            for p in range(2):
                for qc in range(QC):
                    av = [psC.tile([128, 512], F32, name=f"av{i}",
                                   tag=f"av{i}", bufs=2) for i in range(2)]
                    exs = []

                    def av_mms(kt):
                        for i in range(2):
                            nc.tensor.matmul(
                                av[i][:],
                                vS[:, kt, (2 * p + i) * 128:
                                   (2 * p + i + 1) * 128],
                                exs[kt][:, i * 512:(i + 1) * 512],
                                start=(kt == 0), stop=(kt == KT_S - 1))

                    for kt in range(KT_S):
                        sc = psC.tile([128, 1024], F32, name="sc", tag="sc",
                                      bufs=2)
                        nc.tensor.matmul(
                            sc[:, 0:512],
                            kT[0:64, p, kt * 128:(kt + 1) * 128],
                            qT[0:64, p, qc * 512:(qc + 1) * 512],
                            start=True, stop=True, tile_position=(0, 0))
                        nc.tensor.matmul(
                            sc[:, 512:1024],
                            kT[64:128, p, kt * 128:(kt + 1) * 128],
                            qT[64:128, p, qc * 512:(qc + 1) * 512],
                            start=True, stop=True, tile_position=(64, 0))
                        ex = epool.tile([128, 1024], F16, name="ex", tag="ex")
                        nc.scalar.activation(ex[:], sc[:], AF.Exp, scale=0.125)
                        exs.append(ex)
                        # AV lags three kt so exp(k) has more than a full
                        # PE cycle of cover before its consumers issue
                        if kt > 2:
                            av_mms(kt - 3)
                    for ktl in (KT_S - 3, KT_S - 2, KT_S - 1):
                        av_mms(ktl)
                    for i in range(2):
                        sr = npool.tile([1, 512], F32, name="sr", tag=f"sr{i}")
                        nc.vector.tensor_copy(sr[:], av[i][64:65, :])
                        rc = npool.tile([1, 512], F32, name="rc", tag=f"rc{i}")
                        scr = npool.tile([1, 512], F32, name="scr", tag=f"scr{i}")
                        nc.vector.reciprocal_approx_accurate(rc[:], sr[:], scr[:])
                        rb = npool.tile([64, 512], F32, name="rb", tag=f"rb{i}")
                        nc.gpsimd.partition_broadcast(rb[:], rc[:])
                        nc.vector.tensor_tensor(
                            aoT[i * 64:(i + 1) * 64, p,
                                qc * 512:(qc + 1) * 512],
                            av[i][0:64, :], rb[:], op=mybir.AluOpType.mult)

            for k2 in range(2):
                nc.sync.dma_start(
                    wo_t[:, k2, :], woT[k2 * 128:(k2 + 1) * 128, :])
            nc.sync.dma_start(bob_t[:], bob[:])

            # ---- phase D: partial output projection, transposed ----
            # out^T[o, t] = sum_d WoT[d, o] * attn^T[d, t]; Wo stays
            # stationary across the 4 token chunks.
            for ot in range(KT_D):
                acc2 = [
                    psC.tile([128, 512], F32, name=f"acc2{tcx}",
                             tag=("sc" if tcx < 2 else f"av{tcx - 2}"),
                             bufs=2)
                    for tcx in range(4)
                ]
                for k2 in range(2):
                    for tcx in range(4):
                        nc.tensor.matmul(
                            acc2[tcx][:],
                            wo_t[:, k2, ot * 128:(ot + 1) * 128],
                            aoT[:, k2, tcx * 512:(tcx + 1) * 512],
                            start=(k2 == 0), stop=(k2 == 1))
                for tcx in range(4):
                    o_st = opool.tile([128, 512], F16, name="o_st", tag="o_st")
                    nc.vector.tensor_scalar_add(
                        o_st[:], acc2[tcx][:], bob_t[:, ot:ot + 1])
                    nc.sync.dma_start(
                        outT[ot * 128:(ot + 1) * 128,
                             tcx * 512:(tcx + 1) * 512], o_st[:])

    nc.compile()
    return nc


_NC = None


def _get_nc():
    global _NC
    if _NC is None:
        _NC = build_nc()
    return _NC


def kernel(q, k, v, Wq, bq, Wk, bk, Wv, bv, Wo, bo):
    nc = _get_nc()

    q = np.asarray(q, np.float32)
    k = np.asarray(k, np.float32)
    v = np.asarray(v, np.float32)

    xT = {}
    for b in range(B):
        xT[("q", b)] = np.ascontiguousarray(q[b].T)
        xT[("k", b)] = np.ascontiguousarray(k[b].T)
        xT[("v", b)] = np.ascontiguousarray(v[b].T).astype(np.float16)

    WqT = np.ascontiguousarray(np.asarray(Wq, np.float32).T)
    WkT = np.ascontiguousarray(np.asarray(Wk, np.float32).T)
    WvT = np.asarray(Wv, np.float32).T.astype(np.float16)
    WoT = np.asarray(Wo, np.float32).T.astype(np.float16)
    bq = np.asarray(bq, np.float32)
    bk = np.asarray(bk, np.float32)
    bv = np.asarray(bv, np.float32)
    bo = np.asarray(bo, np.float32)

    in_maps = []
    for c in range(NC_):
        b, g = divmod(c, 4)
        sl = slice(g * HG, (g + 1) * HG)
        bqs, bks = bq[sl], bk[sl]
        bqv_a = np.stack(
            [bqs[0:128], bqs[128:256], bks[0:128], bks[128:256]], axis=1)
        bo_a = (bo if g == 0 else np.zeros_like(bo)).reshape(KT_D, 128).T
        in_maps.append({
            "xqT": xT[("q", b)],
            "xkT": xT[("k", b)],
            "xvT": xT[("v", b)],
            "wqT": np.ascontiguousarray(WqT[:, sl]),
            "wkT": np.ascontiguousarray(WkT[:, sl]),
            "wvT": np.ascontiguousarray(WvT[:, sl]),
            "bqv": np.ascontiguousarray(bqv_a),
            "bvb": np.ascontiguousarray(
                np.broadcast_to(bv[sl], (128, HG))),
            "woT": np.ascontiguousarray(WoT[sl, :]),
            "bob": np.ascontiguousarray(bo_a),
        })

    res = run_bass_kernel_spmd(nc, in_maps, list(range(NC_)))

    out = np.empty((B, S, D), np.float32)
    for b in range(B):
        acc = np.zeros((D, S), np.float32)
        for g in range(4):
            acc += res.results[b * 4 + g]["outT"].astype(np.float32)
        out[b] = acc.T
    return out


# revision 15
# speedup vs baseline: 1.0256x; 1.0256x over previous
"""Trainium2 multi-head attention kernel (8 NeuronCores).

Sharding: 2 (batch) x 4 (head-group) grid. Core c handles batch b=c//4 and
heads [4g, 4g+4) where g=c%4 (d_model slice of 256).

Per core:
  1. Q^T,K^T projections [256,2048] (fp32r) and V [2048,256] (fp16) for its
     heads, contraction d_model=1024.
  2. Attention: scores^T = Kh^T.T @ Qh^T per head (two heads packed onto PE
     row-groups via tile_position), exp via ScalarE with fused 1/8 scale, AV
     in fp16 with a ones-augmented V so the softmax denominators fall out of
     the same matmul, then an accurate-reciprocal normalize.
  3. Partial output projection, computed transposed (out^T = WoT.T @ attn^T)
     so the weight stays stationary: [1024, 2048] partial sum in fp16.
Host: all inputs are pre-transposed/sliced per core; the 4 partial outputs of
each batch are summed on host (the unshard step of this tensor-parallel
layout), transposed back and concatenated over batch.
"""
import os
import sys

import numpy as np

for _p in ("/opt/trn_rl_repo", "/root/.axon_site/_ro/trn_rl_repo"):
    if _p not in sys.path:
        sys.path.append(_p)

import concourse.bacc as bacc
import concourse.mybir as mybir
import concourse.tile as tile
from concourse.bass_utils import run_bass_kernel_spmd

F32 = mybir.dt.float32
F32R = mybir.dt.float32r
F16 = mybir.dt.float16

B, S, D, H, DK = 2, 2048, 1024, 16, 64
NC_ = 8
HG = D // 4          # 256: d_model slice per core
KT_D = D // 128      # 8 contraction tiles for projections
KT_S = S // 128      # 16 sequence tiles
QC = S // 512        # 4 query chunks of 512
AF = mybir.ActivationFunctionType

FP16_SCORES = os.environ.get("FP16_SCORES") == "1"
SCORE_DT = F16 if FP16_SCORES else F32R

if os.environ.get("LDW_OPT") == "1":
    import concourse.bass_utils as _bu

    if not getattr(_bu, "_ldw_opt_patched", False):
        _orig_run_command = _bu.run_command

        def _run_command_ldw(cmd, **kw):
            cmd = [c.replace("--enable-ldw-opt=false", "--enable-ldw-opt=true")
                   if isinstance(c, str) else c for c in cmd]
            return _orig_run_command(cmd, **kw)

        _bu.run_command = _run_command_ldw
        _bu._ldw_opt_patched = True


def build_nc():
    nc = bacc.Bacc("TRN2", target_bir_lowering=False, debug=False, num_devices=NC_)

    xqT = nc.dram_tensor("xqT", [D, S], F32R, kind="ExternalInput").ap()
    xkT = nc.dram_tensor("xkT", [D, S], F32R, kind="ExternalInput").ap()
    xvT = nc.dram_tensor("xvT", [D, S], F16, kind="ExternalInput").ap()
    wqT = nc.dram_tensor("wqT", [D, HG], F32R, kind="ExternalInput").ap()
    wkT = nc.dram_tensor("wkT", [D, HG], F32R, kind="ExternalInput").ap()
    wvT = nc.dram_tensor("wvT", [D, HG], F16, kind="ExternalInput").ap()
    bqv = nc.dram_tensor("bqv", [128, 4], F32, kind="ExternalInput").ap()
    bvb = nc.dram_tensor("bvb", [128, HG], F32, kind="ExternalInput").ap()
    woT = nc.dram_tensor("woT", [HG, D], F16, kind="ExternalInput").ap()
    bob = nc.dram_tensor("bob", [128, KT_D], F32, kind="ExternalInput").ap()
    outT = nc.dram_tensor("outT", [D, S], F16, kind="ExternalOutput").ap()

    with tile.TileContext(nc) as tc:
        with (
            tc.tile_pool(name="const", bufs=1) as cpool,
            tc.tile_pool(name="proj", bufs=1) as ppool,
            tc.tile_pool(name="xs", bufs=4) as xpool,
            tc.tile_pool(name="exp", bufs=4) as epool,
            tc.tile_pool(name="nrm", bufs=2) as npool,
            tc.tile_pool(name="ost", bufs=3) as opool,
            tc.tile_pool(name="psC", bufs=1, space="PSUM") as psC,
        ):
            # ---- persistent tiles ----
            wq_t = cpool.tile([128, KT_D, HG], F32R)
            wk_t = cpool.tile([128, KT_D, HG], F32R)
            wv_t = cpool.tile([128, KT_D, HG], F16)
            bqv_t = cpool.tile([128, 4], F32)
            nc.sync.dma_start(bqv_t[:], bqv[:])
            bvb_t = cpool.tile([128, HG], F32)
            bob_t = cpool.tile([128, KT_D], F32)
            wo_t = cpool.tile([128, 2, D], F16)

            qT = ppool.tile([128, 2, S], SCORE_DT)  # [o-part, Mtile, t]
            kT = ppool.tile([128, 2, S], SCORE_DT)
            vS = ppool.tile([128, KT_S, 4 * 128], F16)  # [t-part, t-tile, head*65]
            aoT = ppool.tile([128, 2, S], F16)  # normalized attn out^T

            # ---- phase A: Q^T then K^T projections ----
            # 4 accumulators live (2 Mtiles x 2 qc of a pair), borrowed from
            # the attention pool's tags; lhsT reused across the qc pair.
            for (w_t, w_d, dst, xsrc, xtag, boff) in (
                (wq_t, wqT, qT, xqT, "xq_c", 0),
                (wk_t, wkT, kT, xkT, "xk_c", 2),
            ):
                for kt in range(KT_D):
                    nc.sync.dma_start(
                        w_t[:, kt, :], w_d[kt * 128:(kt + 1) * 128, :])
                for qp in range(QC // 2):
                    acc = [
                        psC.tile([128, 512], F32, name=f"acc{m}{j}",
                                 tag=f"av{j}", bufs=2)
                        for m in range(2) for j in range(2)
                    ]
                    for kt in range(KT_D):
                        x_c = xpool.tile([128, 1024], F32R, name="x_c",
                                         tag=xtag)
                        nc.sync.dma_start(
                            x_c[:], xsrc[kt * 128:(kt + 1) * 128,
                                         qp * 1024:(qp + 1) * 1024])
                        for m in range(2):
                            for j in range(2):
                                nc.tensor.matmul(
                                    acc[m * 2 + j][:],
                                    w_t[:, kt, m * 128:(m + 1) * 128],
                                    x_c[:, j * 512:(j + 1) * 512],
                                    start=(kt == 0), stop=(kt == KT_D - 1))
                    for m in range(2):
                        for j in range(2):
                            qc = qp * 2 + j
                            nc.scalar.activation(
                                dst[:, m, qc * 512:(qc + 1) * 512],
                                acc[m * 2 + j][:],
                                AF.Identity,
                                bias=bqv_t[:, boff + m:boff + m + 1])

            # ---- phase B: V projection (token-major layout, fp16) ----
            # ones columns of vS (softmax denominator trick): fill whole
            # tile with 1.0 once; V writes below overwrite all but col 64.
            nc.gpsimd.memset(vS[:], 1.0)
            for kt in range(KT_D):
                nc.sync.dma_start(
                    wv_t[:, kt, :], wvT[kt * 128:(kt + 1) * 128, :])
            nc.sync.dma_start(bvb_t[:], bvb[:])
            # one accumulator per PSUM bank (interleaved chains in a single
            # bank corrupt each other: start=True clears the whole bank)
            for tc2 in range(KT_S // 2):
                psv = [psC.tile([128, HG], F32, name=f"psv{t}",
                                tag=f"av{t}", bufs=2) for t in range(2)]
                for kt in range(KT_D):
                    xv_c = xpool.tile([128, 256], F16, name="xv_c", tag="xv_c")
                    nc.sync.dma_start(
                        xv_c[:], xvT[kt * 128:(kt + 1) * 128,
                                     tc2 * 256:(tc2 + 1) * 256])
                    for t in range(2):
                        nc.tensor.matmul(
                            psv[t][:],
                            xv_c[:, t * 128:(t + 1) * 128],
                            wv_t[:, kt, :], start=(kt == 0),
                            stop=(kt == KT_D - 1))
                for t in range(2):
                    tg = tc2 * 2 + t
                    for h in range(4):
                        nc.vector.tensor_tensor(
                            vS[:, tg, h * 128:h * 128 + 64],
                            psv[t][:, h * 64:(h + 1) * 64],
                            bvb_t[:, h * 64:(h + 1) * 64],
                            op=mybir.AluOpType.add)

            # ---- phase C: attention ----
            for p in range(2):
                for qc in range(QC):
                    av = [psC.tile([128, 512], F32, name=f"av{i}",
                                   tag=f"av{i}", bufs=2) for i in range(2)]
                    exs = []

                    def av_mms(kt):
                        for i in range(2):
                            nc.tensor.matmul(
                                av[i][:],
                                vS[:, kt, (2 * p + i) * 128:
                                   (2 * p + i + 1) * 128],
                                exs[kt][:, i * 512:(i + 1) * 512],
                                start=(kt == 0), stop=(kt == KT_S - 1))

                    for kt in range(KT_S):
                        sc = psC.tile([128, 1024], F32, name="sc", tag="sc",
                                      bufs=2)
                        nc.tensor.matmul(
                            sc[:, 0:512],
                            kT[0:64, p, kt * 128:(kt + 1) * 128],
                            qT[0:64, p, qc * 512:(qc + 1) * 512],
                            start=True, stop=True, tile_position=(0, 0))
                        nc.tensor.matmul(
                            sc[:, 512:1024],
                            kT[64:128, p, kt * 128:(kt + 1) * 128],
                            qT[64:128, p, qc * 512:(qc + 1) * 512],
                            start=True, stop=True, tile_position=(64, 0))
                        ex = epool.tile([128, 1024], F16, name="ex", tag="ex")
                        nc.scalar.activation(ex[:], sc[:], AF.Exp, scale=0.125)
                        exs.append(ex)
                        # AV lags two kt so exp(k) has a full PE cycle of
                        # cover before its consumers issue
                        if kt > 1:
                            av_mms(kt - 2)
                    av_mms(KT_S - 2)
                    av_mms(KT_S - 1)
                    for i in range(2):
                        sr = npool.tile([1, 512], F32, name="sr", tag=f"sr{i}")
                        nc.vector.tensor_copy(sr[:], av[i][64:65, :])
                        rc = npool.tile([1, 512], F32, name="rc", tag=f"rc{i}")
                        scr = npool.tile([1, 512], F32, name="scr", tag=f"scr{i}")
                        nc.vector.reciprocal_approx_accurate(rc[:], sr[:], scr[:])
                        rb = npool.tile([64, 512], F32, name="rb", tag=f"rb{i}")
                        nc.gpsimd.partition_broadcast(rb[:], rc[:])
                        nc.vector.tensor_tensor(
                            aoT[i * 64:(i + 1) * 64, p,
                                qc * 512:(qc + 1) * 512],
                            av[i][0:64, :], rb[:], op=mybir.AluOpType.mult)

            for k2 in range(2):
                nc.sync.dma_start(
                    wo_t[:, k2, :], woT[k2 * 128:(k2 + 1) * 128, :])
            nc.sync.dma_start(bob_t[:], bob[:])

            # ---- phase D: partial output projection, transposed ----
            # out^T[o, t] = sum_d WoT[d, o] * attn^T[d, t]; Wo stays
            # stationary across the 4 token chunks.
            for ot in range(KT_D):
                acc2 = [
                    psC.tile([128, 512], F32, name=f"acc2{tcx}",
                             tag=("sc" if tcx < 2 else f"av{tcx - 2}"),
                             bufs=2)
                    for tcx in range(4)
                ]
                for k2 in range(2):
                    for tcx in range(4):
                        nc.tensor.matmul(
                            acc2[tcx][:],
                            wo_t[:, k2, ot * 128:(ot + 1) * 128],
                            aoT[:, k2, tcx * 512:(tcx + 1) * 512],
                            start=(k2 == 0), stop=(k2 == 1))
                for tcx in range(4):
                    o_st = opool.tile([128, 512], F16, name="o_st", tag="o_st")
                    nc.vector.tensor_scalar_add(
                        o_st[:], acc2[tcx][:], bob_t[:, ot:ot + 1])
                    nc.sync.dma_start(
                        outT[ot * 128:(ot + 1) * 128,
                             tcx * 512:(tcx + 1) * 512], o_st[:])

    nc.compile()
    return nc


_NC = None


def _get_nc():
    global _NC
    if _NC is None:
        _NC = build_nc()
    return _NC


def kernel(q, k, v, Wq, bq, Wk, bk, Wv, bv, Wo, bo):
    nc = _get_nc()

    q = np.asarray(q, np.float32)
    k = np.asarray(k, np.float32)
    v = np.asarray(v, np.float32)

    xT = {}
    for b in range(B):
        xT[("q", b)] = np.ascontiguousarray(q[b].T)
        xT[("k", b)] = np.ascontiguousarray(k[b].T)
        xT[("v", b)] = np.ascontiguousarray(v[b].T).astype(np.float16)

    WqT = np.ascontiguousarray(np.asarray(Wq, np.float32).T)
    WkT = np.ascontiguousarray(np.asarray(Wk, np.float32).T)
    WvT = np.asarray(Wv, np.float32).T.astype(np.float16)
    WoT = np.asarray(Wo, np.float32).T.astype(np.float16)
    bq = np.asarray(bq, np.float32)
    bk = np.asarray(bk, np.float32)
    bv = np.asarray(bv, np.float32)
    bo = np.asarray(bo, np.float32)

    in_maps = []
    for c in range(NC_):
        b, g = divmod(c, 4)
        sl = slice(g * HG, (g + 1) * HG)
        bqs, bks = bq[sl], bk[sl]
        bqv_a = np.stack(
            [bqs[0:128], bqs[128:256], bks[0:128], bks[128:256]], axis=1)
        bo_a = (bo if g == 0 else np.zeros_like(bo)).reshape(KT_D, 128).T
        in_maps.append({
            "xqT": xT[("q", b)],
            "xkT": xT[("k", b)],
            "xvT": xT[("v", b)],
            "wqT": np.ascontiguousarray(WqT[:, sl]),
            "wkT": np.ascontiguousarray(WkT[:, sl]),
            "wvT": np.ascontiguousarray(WvT[:, sl]),
            "bqv": np.ascontiguousarray(bqv_a),
            "bvb": np.ascontiguousarray(
                np.broadcast_to(bv[sl], (128, HG))),
            "woT": np.ascontiguousarray(WoT[sl, :]),
            "bob": np.ascontiguousarray(bo_a),
        })

    res = run_bass_kernel_spmd(nc, in_maps, list(range(NC_)))

    out = np.empty((B, S, D), np.float32)
    for b in range(B):
        acc = np.zeros((D, S), np.float32)
        for g in range(4):
            acc += res.results[b * 4 + g]["outT"].astype(np.float32)
        out[b] = acc.T
    return out
